# revision 1
# baseline (speedup 1.0000x reference)
"""TRN2 Bass kernel for nn_Attention (cross-attention, Tq=2, Tk=5, B=16384, D=512).

Math reformulation (exact):
    q~ = h @ W_A,        W_A  = Wq @ Wk^T          (host-precomputed, tiny)
    logits[b,i,j] = q~[b,i,:] . e[b,j,:]           (DVE dots, fp32 accum)
    ex = exp(logits - max)                          (Act)
    ctxu[b,i,:] = sum_j ex[b,i,j] * e[b,j,:]       (PE: diag(ex) matmuls, PSUM accum)
    ctx = ctxu / sum_j ex                           (folded into Act PSUM->SBUF copy)
    out = h @ Wd1 + ctx @ W_vd,  W_vd = Wv @ Wd2   (host-precomputed, tiny)

Per-batch weighted sums run on the PE via diagonal stationary matrices:
    matmul(psum, lhsT=diag(ex_ij), rhs=e_j)  accumulates ex_ij[b]*e[b,j,:] per lane.
diag(ex_ij) is a single-scalar 4x-mode tensor_scalar op on a fp16 identity.
Softmax normalization rides the Act-engine copy (per-partition scale = 1/sum).

Sharding: pure data parallel over batch, 2048 per core x 8 cores.
Host marshals e to batch-major [B, Tk, D] fp16 and h to block-transposed
lhsT layout [NT, P(d), DC, Tq, P(b)] fp16. Output fp16, upcast on host.
Main loop is a 3-stage software pipeline (A: loads+q~ | B: dots+max+exp |
C: recip+diag+ctx+transpose+out) so the DVE never stalls on Act's EXP.
"""

import numpy as np

import concourse.bass as bass
import concourse.mybir as mybir
import concourse.tile as tile
from concourse import bacc
from concourse.bass_utils import run_bass_kernel_spmd
from concourse.masks import make_identity

F32 = mybir.dt.float32
F16 = mybir.dt.float16
MUL = mybir.AluOpType.mult
ADD = mybir.AluOpType.add
BYP = mybir.AluOpType.bypass

TQ, TK, B, D = 2, 5, 16384, 512
NCORES = 8
BL = B // NCORES          # 2048 batch per core
P = 128                   # partition tile
NT = BL // P              # 16 batch tiles per core
DC = D // P               # 4 contraction chunks

_CACHED = {}


def build():
    nc = bacc.Bacc("TRN2", target_bir_lowering=False, debug=False)

    e_d = nc.dram_tensor("enc", [BL, TK, D], F16, kind="ExternalInput")
    ht_d = nc.dram_tensor("hT", [NT, P, DC, TQ, P], F16, kind="ExternalInput")
    wqk_d = nc.dram_tensor("Wqk", [P, DC, D], F16, kind="ExternalInput")
    wd1_d = nc.dram_tensor("Wd1", [P, DC, D], F16, kind="ExternalInput")
    wvd_d = nc.dram_tensor("Wvd", [P, DC, D], F16, kind="ExternalInput")
    o_d = nc.dram_tensor("out", [BL, TQ, D], F16, kind="ExternalOutput")

    e_r = e_d.ap()
    o_r = o_d.ap()

    with tile.TileContext(nc) as tc:
        with (
            tc.tile_pool(name="wgt", bufs=1) as wgt,
            tc.tile_pool(name="io", bufs=8) as io,
            tc.tile_pool(name="qp", bufs=4) as qp,
            tc.tile_pool(name="work", bufs=4) as work,
            tc.tile_pool(name="small", bufs=4) as small,
            tc.tile_pool(name="scr", bufs=2) as scr,
            tc.tile_pool(name="obp", bufs=2) as obp,
            tc.tile_pool(name="psq", bufs=2, space="PSUM") as psq,   # [P,D]f32 1bk x2
            tc.tile_pool(name="psc", bufs=2, space="PSUM") as psc,   # [P,D]f32 1bk x2
            tc.tile_pool(name="pso", bufs=2, space="PSUM") as pso,   # [P,D]f32 1bk x2
            tc.tile_pool(name="pst", bufs=2, space="PSUM") as pst,   # [P,8,P]f16 1bk x2
        ):
            ident = wgt.tile([P, P], F16)
            make_identity(nc, ident)

            wqk = wgt.tile([P, DC, D], F16, tag="wqk")
            wd1 = wgt.tile([P, DC, D], F16, tag="wd1")
            wvd = wgt.tile([P, DC, D], F16, tag="wvd")
            nc.gpsimd.dma_start(out=wqk, in_=wqk_d.ap())
            nc.gpsimd.dma_start(out=wd1, in_=wd1_d.ap())
            nc.gpsimd.dma_start(out=wvd, in_=wvd_d.ap())

            # ================= 3-stage software-pipelined loop =================
            def stage_a(t):
                bsl = slice(t * P, (t + 1) * P)
                hT = io.tile([P, DC, TQ, P], F16, tag="hT", name=f"hT{t}")
                nc.sync.dma_start(out=hT, in_=ht_d.ap()[t])
                en = io.tile([P, TK, D], F16, tag="en", name=f"en{t}")
                nc.sync.dma_start(out=en, in_=e_r[bsl])
                st = dict(t=t, en=en, hT=hT)

                # q~ = h @ W_A   [P, TQ, D]
                qn = qp.tile([P, TQ, D], F16, tag="qn", name=f"qn{t}")
                for i in range(TQ):
                    pq = psq.tile([P, D], F32, tag="pq", name=f"pq{t}_{i}")
                    for c in range(DC):
                        nc.tensor.matmul(
                            pq, hT[:, c, i, :], wqk[:, c, :],
                            start=(c == 0), stop=(c == DC - 1))
                    nc.scalar.copy(qn[:, i, :], pq)
                st["qn"] = qn
                return st

            def stage_b(st):
                t, en, qn = st["t"], st["en"], st["qn"]

                # logits[b,i,j] = q~_i . e_j  (DVE 1x dots, fp32 accumulator)
                lg = small.tile([P, TQ, TK], F32, tag="lg", name=f"lg{t}")
                dump = scr.tile([P, D], F16, tag="dump", name=f"du{t}")
                for i in range(TQ):
                    for j in range(TK):
                        nc.vector.scalar_tensor_tensor(
                            out=dump,
                            in0=qn[:, i, :], scalar=1.0, in1=en[:, j, :],
                            op0=BYP, op1=MUL,
                            accum_out=lg[:, i, j:j + 1])

                nmx = small.tile([P, TQ], F32, tag="nmx", name=f"nm{t}")
                nc.vector.tensor_reduce(
                    out=nmx, in_=lg, axis=mybir.AxisListType.X,
                    op=mybir.AluOpType.max, negate=True)
                pr = small.tile([P, TQ, TK], F32, tag="pr", name=f"pr{t}")
                sm = small.tile([P, TQ], F32, tag="sm", name=f"sm{t}")
                for i in range(TQ):
                    nc.scalar.activation(
                        out=pr[:, i, :], in_=lg[:, i, :],
                        func=mybir.ActivationFunctionType.Exp,
                        bias=nmx[:, i:i + 1],
                        accum_out=sm[:, i:i + 1])
                st.update(pr=pr, sm=sm)
                return st

            def stage_c1(st):
                t, en, pr, sm = st["t"], st["en"], st["pr"], st["sm"]

                rs = small.tile([P, TQ], F32, tag="rs", name=f"rs{t}")
                nc.vector.reciprocal(rs, sm)

                # diag(ex_ij) = ident * ex_ij (i=0 on DVE 4x, i=1 on Act
                # scale). The (i=0, j=TK-1) term moves to a DVE accumulate
                # below to shave one N=512 matmul off the saturated PE.
                dg = work.tile([P, TQ, TK, P], F16, tag="dg", name=f"dg{t}")
                for j in range(TK - 1):
                    nc.vector.tensor_scalar_mul(
                        dg[:, 0, j, :], ident, pr[:, 0, j:j + 1])
                for j in range(TK):
                    nc.scalar.mul(dg[:, 1, j, :], ident, pr[:, 1, j:j + 1])

                # ctxu_i = sum_j diag(ex_ij) @ e_j   (PE, PSUM accumulation)
                # normalize during PSUM->SBUF copy: ctx_i = ctxu_i * (1/sum_i)
                cx = work.tile([P, TQ, D], F16, tag="cx", name=f"cx{t}")
                nj = {0: TK - 1, 1: TK}
                for i in range(TQ):
                    pc = psc.tile([P, D], F32, tag="pc", name=f"pc{t}_{i}")
                    for j in range(nj[i]):
                        nc.tensor.matmul(
                            pc, dg[:, i, j, :], en[:, j, :],
                            start=(j == 0), stop=(j == nj[i] - 1))
                    nc.scalar.mul(cx[:, i, :], pc, rs[:, i:i + 1])
                # cx_0 += (ex/sum)[0,TK-1] * e_{TK-1}   (DVE)
                aw = small.tile([P, 1], F32, tag="aw", name=f"aw{t}")
                nc.vector.tensor_scalar_mul(aw, pr[:, 0, TK - 1:TK], rs[:, 0:1])
                nc.vector.scalar_tensor_tensor(
                    out=cx[:, 0, :], in0=en[:, TK - 1, :], scalar=aw,
                    in1=cx[:, 0, :], op0=MUL, op1=ADD)
                st["cx"] = cx
                return st

            def stage_c2(st):
                t, hT, cx = st["t"], st["hT"], st["cx"]
                bsl = slice(t * P, (t + 1) * P)

                # transpose ctx -> cT [P(d), DC, TQ, P(b)]
                pt = pst.tile([P, TQ * DC, P], F16, tag="pt", name=f"pt{t}")
                for i in range(TQ):
                    for c in range(DC):
                        nc.tensor.transpose(
                            pt[:, i * DC + c, :],
                            cx[:, i, c * P:(c + 1) * P], ident)
                cT = work.tile([P, DC, TQ, P], F16, tag="cT", name=f"cT{t}")
                nc.scalar.copy(cT, pt.rearrange("p (i c) b -> p c i b", i=TQ))

                # out_i = h_i @ Wd1 + ctx_i @ Wvd
                ob = obp.tile([P, TQ, D], F16, tag="ob", name=f"ob{t}")
                for i in range(TQ):
                    po = pso.tile([P, D], F32, tag="po", name=f"po{t}_{i}")
                    for c in range(DC):
                        nc.tensor.matmul(po, hT[:, c, i, :], wd1[:, c, :],
                                         start=(c == 0), stop=False)
                    for c in range(DC):
                        nc.tensor.matmul(po, cT[:, c, i, :], wvd[:, c, :],
                                         start=False, stop=(c == DC - 1))
                    nc.scalar.copy(ob[:, i, :], po)
                nc.sync.dma_start(out=o_r[bsl], in_=ob)

            stA, stB, stC = {}, {}, {}
            for tt in range(NT + 3):
                if tt < NT:
                    stA[tt] = stage_a(tt)
                if 1 <= tt < NT + 1:
                    stB[tt - 1] = stage_b(stA.pop(tt - 1))
                if 2 <= tt < NT + 2:
                    stC[tt - 2] = stage_c1(stB.pop(tt - 2))
                if tt >= 3:
                    stage_c2(stC.pop(tt - 3))

    nc.compile()
    return nc


def _pack_w(w):
    # [D, D] fp32 -> [P, DC, D] fp16 with contraction dim on partitions
    return np.ascontiguousarray(
        w.reshape(DC, P, D).transpose(1, 0, 2).astype(np.float16))


def kernel(h, enc_out, Wq, Wk, Wv, Wdown, _trace=False):
    h = np.ascontiguousarray(h, dtype=np.float32)
    enc_out = np.ascontiguousarray(enc_out, dtype=np.float32)
    Wq = np.ascontiguousarray(Wq, dtype=np.float32)
    Wk = np.ascontiguousarray(Wk, dtype=np.float32)
    Wv = np.ascontiguousarray(Wv, dtype=np.float32)
    Wdown = np.ascontiguousarray(Wdown, dtype=np.float32)

    if "nc" not in _CACHED:
        _CACHED["nc"] = build()
    nc = _CACHED["nc"]

    w_a = _pack_w(Wq @ Wk.T)
    w_d1 = _pack_w(Wdown[:D])
    w_vd = _pack_w(Wv @ Wdown[D:])

    h16 = h.astype(np.float16)
    e16 = enc_out.astype(np.float16)
    e_bm = np.ascontiguousarray(e16.transpose(1, 0, 2))        # [B, TK, D]
    # block-transposed lhsT tiles: [core][t][p(d%128)][c][i][p(b%128)]
    hT_bm = np.ascontiguousarray(
        h16.reshape(TQ, NCORES, NT, P, DC, P).transpose(1, 2, 5, 4, 0, 3))
    in_maps = []
    for c in range(NCORES):
        sl = slice(c * BL, (c + 1) * BL)
        in_maps.append({
            "enc": e_bm[sl],
            "hT": hT_bm[c],
            "Wqk": w_a, "Wd1": w_d1, "Wvd": w_vd,
        })

    res = run_bass_kernel_spmd(nc, in_maps, list(range(NCORES)), trace=_trace)
    out_bm = np.concatenate([r["out"] for r in res.results], axis=0)  # [B, TQ, D]
    out = np.ascontiguousarray(out_bm.transpose(1, 0, 2))
    if _trace:
        kernel.last_result = res
    return out.astype(np.float32)



# revision 2
# speedup vs baseline: 1.0139x; 1.0139x over previous
"""TRN2 Bass kernel for nn_Attention (cross-attention, Tq=2, Tk=5, B=16384, D=512).

Math reformulation (exact):
    q~ = h @ W_A,        W_A  = Wq @ Wk^T          (host-precomputed, tiny)
    logits[b,i,j] = q~[b,i,:] . e[b,j,:]           (DVE dots, fp32 accum)
    ex = exp(logits - max)                          (Act)
    ctxu[b,i,:] = sum_j ex[b,i,j] * e[b,j,:]       (PE: diag(ex) matmuls, PSUM accum)
    ctx = ctxu / sum_j ex                           (folded into Act PSUM->SBUF copy)
    out = h @ Wd1 + ctx @ W_vd,  W_vd = Wv @ Wd2   (host-precomputed, tiny)

Per-batch weighted sums run on the PE via diagonal stationary matrices:
    matmul(psum, lhsT=diag(ex_ij), rhs=e_j)  accumulates ex_ij[b]*e[b,j,:] per lane.
diag(ex_ij) is a single-scalar 4x-mode tensor_scalar op on a fp16 identity.
Softmax normalization rides the Act-engine copy (per-partition scale = 1/sum).

Sharding: pure data parallel over batch, 2048 per core x 8 cores.
Host marshals e to batch-major [B, Tk, D] fp16 and h to block-transposed
lhsT layout [NT, P(d), DC, Tq, P(b)] fp16. Output fp16, upcast on host.
Main loop is a 3-stage software pipeline (A: loads+q~ | B: dots+max+exp |
C: recip+diag+ctx+transpose+out) so the DVE never stalls on Act's EXP.

Execution tuning: PE warmup matmuls during the initial DMA wait (HAM
clock-gate reaches 2.4 GHz before the real stream), weight DMAs ordered
by first use with wqk split, diag-build DVE ops emitted ahead of the
dot block, per-i output evac+store to shorten the tail.
"""

import numpy as np

import concourse.bass as bass
import concourse.mybir as mybir
import concourse.tile as tile
from concourse import bacc
from concourse.bass_utils import run_bass_kernel_spmd
from concourse.masks import make_identity

F32 = mybir.dt.float32
F16 = mybir.dt.float16
MUL = mybir.AluOpType.mult
ADD = mybir.AluOpType.add
BYP = mybir.AluOpType.bypass

TQ, TK, B, D = 2, 5, 16384, 512
NCORES = 8
BL = B // NCORES          # 2048 batch per core
P = 128                   # partition tile
NT = BL // P              # 16 batch tiles per core
DC = D // P               # 4 contraction chunks

_CACHED = {}


def build():
    nc = bacc.Bacc("TRN2", target_bir_lowering=False, debug=False)

    e_d = nc.dram_tensor("enc", [BL, TK, D], F16, kind="ExternalInput")
    ht_d = nc.dram_tensor("hT", [NT, P, DC, TQ, P], F16, kind="ExternalInput")
    wqk_d = nc.dram_tensor("Wqk", [P, DC, D], F16, kind="ExternalInput")
    wd1_d = nc.dram_tensor("Wd1", [P, DC, D], F16, kind="ExternalInput")
    wvd_d = nc.dram_tensor("Wvd", [P, DC, D], F16, kind="ExternalInput")
    o_d = nc.dram_tensor("out", [BL, TQ, D], F16, kind="ExternalOutput")

    e_r = e_d.ap()
    o_r = o_d.ap()

    with tile.TileContext(nc) as tc:
        with (
            tc.tile_pool(name="wgt", bufs=1) as wgt,
            tc.tile_pool(name="io", bufs=8) as io,
            tc.tile_pool(name="qp", bufs=4) as qp,
            tc.tile_pool(name="work", bufs=4) as work,
            tc.tile_pool(name="small", bufs=4) as small,
            tc.tile_pool(name="scr", bufs=2) as scr,
            tc.tile_pool(name="obp", bufs=2) as obp,
            tc.tile_pool(name="psq", bufs=2, space="PSUM") as psq,   # [P,D]f32 1bk x2
            tc.tile_pool(name="psc", bufs=2, space="PSUM") as psc,   # [P,D]f32 1bk x2
            tc.tile_pool(name="pso", bufs=2, space="PSUM") as pso,   # [P,D]f32 1bk x2
            tc.tile_pool(name="pst", bufs=2, space="PSUM") as pst,   # [P,8,P]f16 1bk x2
        ):
            # Weights ordered by first use: wqk (q~) first and split in two
            # so the c=0/1 chain can start sooner; identity (needed by diag,
            # step 2) before wd1/wvd (needed by out, step 3) on gpsimd.
            wqk = wgt.tile([P, DC, D], F16, tag="wqk")
            wd1 = wgt.tile([P, DC, D], F16, tag="wd1")
            wvd = wgt.tile([P, DC, D], F16, tag="wvd")
            nc.sync.dma_start(out=wqk[:, :2, :], in_=wqk_d.ap()[:, :2, :])
            nc.sync.dma_start(out=wqk[:, 2:, :], in_=wqk_d.ap()[:, 2:, :])
            ident = wgt.tile([P, P], F16)
            make_identity(nc, ident)
            nc.gpsimd.dma_start(out=wd1, in_=wd1_d.ap())
            nc.gpsimd.dma_start(out=wvd, in_=wvd_d.ap())

            # PE warmup: dummy matmuls during the initial DMA wait trip the
            # HAM activity monitor to K=8/8 (2.4 GHz) before the real stream
            # begins; otherwise the first ~3.4us of real matmuls run at
            # half clock. Zeros tile -> results never read.
            warm = wgt.tile([P, D], F16, tag="warm")
            nc.vector.memset(warm, 0.0)
            for w in range(14):
                pw = psq.tile([P, D], F32, tag="pq", name=f"warm{w}")
                nc.tensor.matmul(pw, warm[:, :P], warm, start=True, stop=True)

            # ================= 3-stage software-pipelined loop =================
            def stage_a(t):
                bsl = slice(t * P, (t + 1) * P)
                hT = io.tile([P, DC, TQ, P], F16, tag="hT", name=f"hT{t}")
                nc.sync.dma_start(out=hT, in_=ht_d.ap()[t])
                en = io.tile([P, TK, D], F16, tag="en", name=f"en{t}")
                nc.sync.dma_start(out=en, in_=e_r[bsl])
                st = dict(t=t, en=en, hT=hT)

                # q~ = h @ W_A   [P, TQ, D]
                qn = qp.tile([P, TQ, D], F16, tag="qn", name=f"qn{t}")
                for i in range(TQ):
                    pq = psq.tile([P, D], F32, tag="pq", name=f"pq{t}_{i}")
                    for c in range(DC):
                        nc.tensor.matmul(
                            pq, hT[:, c, i, :], wqk[:, c, :],
                            start=(c == 0), stop=(c == DC - 1))
                    nc.scalar.copy(qn[:, i, :], pq)
                st["qn"] = qn
                return st

            def stage_b(st):
                t, en, qn = st["t"], st["en"], st["qn"]

                # logits[b,i,j] = q~_i . e_j  (DVE 1x dots, fp32 accumulator)
                lg = small.tile([P, TQ, TK], F32, tag="lg", name=f"lg{t}")
                dump = scr.tile([P, D], F16, tag="dump", name=f"du{t}")
                for i in range(TQ):
                    for j in range(TK):
                        nc.vector.scalar_tensor_tensor(
                            out=dump,
                            in0=qn[:, i, :], scalar=1.0, in1=en[:, j, :],
                            op0=BYP, op1=MUL,
                            accum_out=lg[:, i, j:j + 1])

                nmx = small.tile([P, TQ], F32, tag="nmx", name=f"nm{t}")
                nc.vector.tensor_reduce(
                    out=nmx, in_=lg, axis=mybir.AxisListType.X,
                    op=mybir.AluOpType.max, negate=True)
                pr = small.tile([P, TQ, TK], F32, tag="pr", name=f"pr{t}")
                sm = small.tile([P, TQ], F32, tag="sm", name=f"sm{t}")
                for i in range(TQ):
                    nc.scalar.activation(
                        out=pr[:, i, :], in_=lg[:, i, :],
                        func=mybir.ActivationFunctionType.Exp,
                        bias=nmx[:, i:i + 1],
                        accum_out=sm[:, i:i + 1])
                st.update(pr=pr, sm=sm)
                return st

            def stage_c1(st):
                t, en, pr, sm = st["t"], st["en"], st["pr"], st["sm"]

                rs = small.tile([P, TQ], F32, tag="rs", name=f"rs{t}")
                nc.vector.reciprocal(rs, sm)

                # diag(ex_ij) = ident * ex_ij (i=0 on DVE 4x, i=1 on Act
                # scale). The (i=0, j=TK-1) term moves to a DVE accumulate
                # below to shave one N=512 matmul off the saturated PE.
                dg = work.tile([P, TQ, TK, P], F16, tag="dg", name=f"dg{t}")
                for j in range(TK - 1):
                    nc.vector.tensor_scalar_mul(
                        dg[:, 0, j, :], ident, pr[:, 0, j:j + 1])
                for j in range(TK):
                    nc.scalar.mul(dg[:, 1, j, :], ident, pr[:, 1, j:j + 1])

                # ctxu_i = sum_j diag(ex_ij) @ e_j   (PE, PSUM accumulation)
                # normalize during PSUM->SBUF copy: ctx_i = ctxu_i * (1/sum_i)
                cx = work.tile([P, TQ, D], F16, tag="cx", name=f"cx{t}")
                nj = {0: TK - 1, 1: TK}
                for i in range(TQ):
                    pc = psc.tile([P, D], F32, tag="pc", name=f"pc{t}_{i}")
                    for j in range(nj[i]):
                        nc.tensor.matmul(
                            pc, dg[:, i, j, :], en[:, j, :],
                            start=(j == 0), stop=(j == nj[i] - 1))
                    nc.scalar.mul(cx[:, i, :], pc, rs[:, i:i + 1])
                # cx_0 += (ex/sum)[0,TK-1] * e_{TK-1}   (DVE)
                aw = small.tile([P, 1], F32, tag="aw", name=f"aw{t}")
                nc.vector.tensor_scalar_mul(aw, pr[:, 0, TK - 1:TK], rs[:, 0:1])
                nc.vector.scalar_tensor_tensor(
                    out=cx[:, 0, :], in0=en[:, TK - 1, :], scalar=aw,
                    in1=cx[:, 0, :], op0=MUL, op1=ADD)
                st["cx"] = cx
                return st

            def stage_c2(st):
                t, hT, cx = st["t"], st["hT"], st["cx"]
                bsl = slice(t * P, (t + 1) * P)

                # transpose ctx -> cT [P(d), DC, TQ, P(b)]
                pt = pst.tile([P, TQ * DC, P], F16, tag="pt", name=f"pt{t}")
                for i in range(TQ):
                    for c in range(DC):
                        nc.tensor.transpose(
                            pt[:, i * DC + c, :],
                            cx[:, i, c * P:(c + 1) * P], ident)
                cT = work.tile([P, DC, TQ, P], F16, tag="cT", name=f"cT{t}")
                nc.scalar.copy(cT, pt.rearrange("p (i c) b -> p c i b", i=TQ))

                # out_i = h_i @ Wd1 + ctx_i @ Wvd; evac+store per i so the
                # final store isn't serialized behind both chains (tail)
                ob = obp.tile([P, TQ, D], F16, tag="ob", name=f"ob{t}")
                for i in range(TQ):
                    po = pso.tile([P, D], F32, tag="po", name=f"po{t}_{i}")
                    for c in range(DC):
                        nc.tensor.matmul(po, hT[:, c, i, :], wd1[:, c, :],
                                         start=(c == 0), stop=False)
                    for c in range(DC):
                        nc.tensor.matmul(po, cT[:, c, i, :], wvd[:, c, :],
                                         start=False, stop=(c == DC - 1))
                    nc.scalar.copy(ob[:, i, :], po)
                    nc.sync.dma_start(out=o_r[bsl, i, :], in_=ob[:, i, :])

            # Emit C1 before B each step: C1's small DVE ops (recip, diag
            # builds) feed this step's PE diag matmuls; queueing them ahead
            # of B's 6us dot block removes a PE stall window.
            stA, stB, stC = {}, {}, {}
            for tt in range(NT + 3):
                if tt < NT:
                    stA[tt] = stage_a(tt)
                if 2 <= tt < NT + 2:
                    stC[tt - 2] = stage_c1(stB.pop(tt - 2))
                if 1 <= tt < NT + 1:
                    stB[tt - 1] = stage_b(stA.pop(tt - 1))
                if tt >= 3:
                    stage_c2(stC.pop(tt - 3))

    nc.compile()
    return nc


def _pack_w(w):
    # [D, D] fp32 -> [P, DC, D] fp16 with contraction dim on partitions
    return np.ascontiguousarray(
        w.reshape(DC, P, D).transpose(1, 0, 2).astype(np.float16))


def kernel(h, enc_out, Wq, Wk, Wv, Wdown, _trace=False):
    h = np.ascontiguousarray(h, dtype=np.float32)
    enc_out = np.ascontiguousarray(enc_out, dtype=np.float32)
    Wq = np.ascontiguousarray(Wq, dtype=np.float32)
    Wk = np.ascontiguousarray(Wk, dtype=np.float32)
    Wv = np.ascontiguousarray(Wv, dtype=np.float32)
    Wdown = np.ascontiguousarray(Wdown, dtype=np.float32)

    if "nc" not in _CACHED:
        _CACHED["nc"] = build()
    nc = _CACHED["nc"]

    w_a = _pack_w(Wq @ Wk.T)
    w_d1 = _pack_w(Wdown[:D])
    w_vd = _pack_w(Wv @ Wdown[D:])

    h16 = h.astype(np.float16)
    e16 = enc_out.astype(np.float16)
    e_bm = np.ascontiguousarray(e16.transpose(1, 0, 2))        # [B, TK, D]
    # block-transposed lhsT tiles: [core][t][p(d%128)][c][i][p(b%128)]
    hT_bm = np.ascontiguousarray(
        h16.reshape(TQ, NCORES, NT, P, DC, P).transpose(1, 2, 5, 4, 0, 3))
    in_maps = []
    for c in range(NCORES):
        sl = slice(c * BL, (c + 1) * BL)
        in_maps.append({
            "enc": e_bm[sl],
            "hT": hT_bm[c],
            "Wqk": w_a, "Wd1": w_d1, "Wvd": w_vd,
        })

    res = run_bass_kernel_spmd(nc, in_maps, list(range(NCORES)), trace=_trace)
    out_bm = np.concatenate([r["out"] for r in res.results], axis=0)  # [B, TQ, D]
    out = np.ascontiguousarray(out_bm.transpose(1, 0, 2))
    if _trace:
        kernel.last_result = res
    return out.astype(np.float32)



# revision 3
# speedup vs baseline: 1.0362x; 1.0220x over previous
"""TRN2 Bass kernel for nn_Attention (cross-attention, Tq=2, Tk=5, B=16384, D=512).

Math reformulation (exact):
    q~ = h @ W_A,        W_A  = Wq @ Wk^T          (host-precomputed, tiny)
    logits[b,i,j] = q~[b,i,:] . e[b,j,:]           (DVE dots, fp32 accum)
    ex = exp(logits - max)                          (Act)
    ctxu[b,i,:] = sum_j ex[b,i,j] * e[b,j,:]       (PE: diag(ex) matmuls, PSUM accum)
    ctx = ctxu / sum_j ex                           (folded into Act PSUM->SBUF copy)
    out = h @ Wd1 + ctx @ W_vd,  W_vd = Wv @ Wd2   (host-precomputed, tiny)

Per-batch weighted sums run on the PE via diagonal stationary matrices:
    matmul(psum, lhsT=diag(ex_ij), rhs=e_j)  accumulates ex_ij[b]*e[b,j,:] per lane.
diag(ex_ij) is a single-scalar 4x-mode tensor_scalar op on a fp16 identity.
Softmax normalization rides the Act-engine copy (per-partition scale = 1/sum).

Sharding: pure data parallel over batch, 2048 per core x 8 cores.
Host marshals e to batch-major [B, Tk, D] fp16 and h to block-transposed
lhsT layout [NT, P(d), DC, Tq, P(b)] fp16. Output fp16, upcast on host.
Main loop is a 3-stage software pipeline (A: loads+q~ | B: dots+max+exp |
C: recip+diag+ctx+transpose+out) so the DVE never stalls on Act's EXP.

Execution tuning: q~ stays in PSUM and the 1x STT dots read it directly
(no Act evacuation, Act off the q~->dots critical path); PE warmup
matmuls during the initial DMA wait (HAM clock-gate reaches 2.4 GHz
before the real stream); weight DMAs ordered by first use with wqk
split; diag-build DVE ops emitted ahead of the dot block; per-i output
evac+store to shorten the tail.
"""

import numpy as np

import concourse.bass as bass
import concourse.mybir as mybir
import concourse.tile as tile
from concourse import bacc
from concourse.bass_utils import run_bass_kernel_spmd
from concourse.masks import make_identity

F32 = mybir.dt.float32
F16 = mybir.dt.float16
MUL = mybir.AluOpType.mult
ADD = mybir.AluOpType.add
BYP = mybir.AluOpType.bypass

TQ, TK, B, D = 2, 5, 16384, 512
NCORES = 8
BL = B // NCORES          # 2048 batch per core
P = 128                   # partition tile
NT = BL // P              # 16 batch tiles per core
DC = D // P               # 4 contraction chunks

_CACHED = {}


def build():
    nc = bacc.Bacc("TRN2", target_bir_lowering=False, debug=False)

    e_d = nc.dram_tensor("enc", [BL, TK, D], F16, kind="ExternalInput")
    ht_d = nc.dram_tensor("hT", [NT, P, DC, TQ, P], F16, kind="ExternalInput")
    wqk_d = nc.dram_tensor("Wqk", [P, DC, D], F16, kind="ExternalInput")
    wd1_d = nc.dram_tensor("Wd1", [P, DC, D], F16, kind="ExternalInput")
    wvd_d = nc.dram_tensor("Wvd", [P, DC, D], F16, kind="ExternalInput")
    o_d = nc.dram_tensor("out", [BL, TQ, D], F16, kind="ExternalOutput")

    e_r = e_d.ap()
    o_r = o_d.ap()

    with tile.TileContext(nc) as tc:
        with (
            tc.tile_pool(name="wgt", bufs=1) as wgt,
            tc.tile_pool(name="io", bufs=8) as io,
            tc.tile_pool(name="qp", bufs=4) as qp,
            tc.tile_pool(name="work", bufs=4) as work,
            tc.tile_pool(name="small", bufs=4) as small,
            tc.tile_pool(name="scr", bufs=2) as scr,
            tc.tile_pool(name="obp", bufs=2) as obp,
            tc.tile_pool(name="psq", bufs=2, space="PSUM") as psq,   # [P,D]f32 1bk x2
            tc.tile_pool(name="psc", bufs=2, space="PSUM") as psc,   # [P,D]f32 1bk x2
            tc.tile_pool(name="pso", bufs=2, space="PSUM") as pso,   # [P,D]f32 1bk x2
            tc.tile_pool(name="pst", bufs=2, space="PSUM") as pst,   # [P,8,P]f16 1bk x2
        ):
            # Weights ordered by first use: wqk (q~) first and split in two
            # so the c=0/1 chain can start sooner; identity (needed by diag,
            # step 2) before wd1/wvd (needed by out, step 3) on gpsimd.
            wqk = wgt.tile([P, DC, D], F16, tag="wqk")
            wd1 = wgt.tile([P, DC, D], F16, tag="wd1")
            wvd = wgt.tile([P, DC, D], F16, tag="wvd")
            nc.sync.dma_start(out=wqk[:, :2, :], in_=wqk_d.ap()[:, :2, :])
            nc.sync.dma_start(out=wqk[:, 2:, :], in_=wqk_d.ap()[:, 2:, :])
            ident = wgt.tile([P, P], F16)
            make_identity(nc, ident)
            nc.gpsimd.dma_start(out=wd1, in_=wd1_d.ap())
            nc.gpsimd.dma_start(out=wvd, in_=wvd_d.ap())

            # PE warmup: dummy matmuls during the initial DMA wait trip the
            # HAM activity monitor to K=8/8 (2.4 GHz) before the real stream
            # begins; otherwise the first ~3.4us of real matmuls run at
            # half clock. Zeros tile -> results never read.
            warm = wgt.tile([P, D], F16, tag="warm")
            nc.vector.memset(warm, 0.0)
            for w in range(14):
                pw = psq.tile([P, D], F32, tag="pq", name=f"warm{w}")
                nc.tensor.matmul(pw, warm[:, :P], warm, start=True, stop=True)

            # ================= 3-stage software-pipelined loop =================
            def stage_a(t):
                bsl = slice(t * P, (t + 1) * P)
                hT = io.tile([P, DC, TQ, P], F16, tag="hT", name=f"hT{t}")
                nc.sync.dma_start(out=hT, in_=ht_d.ap()[t])
                en = io.tile([P, TK, D], F16, tag="en", name=f"en{t}")
                nc.sync.dma_start(out=en, in_=e_r[bsl])
                st = dict(t=t, en=en, hT=hT)

                # q~ = h @ W_A stays in PSUM: the STT dots are 1x anyway,
                # so they read it directly (+52ns/dot) and the Act
                # evacuation (2x570ns) plus its dependency hop disappear.
                pqs = []
                for i in range(TQ):
                    pq = psq.tile([P, D], F32, tag="pq", name=f"pq{t}_{i}")
                    for c in range(DC):
                        nc.tensor.matmul(
                            pq, hT[:, c, i, :], wqk[:, c, :],
                            start=(c == 0), stop=(c == DC - 1))
                    pqs.append(pq)
                st["pq"] = pqs
                return st

            def stage_b(st):
                t, en, pqs = st["t"], st["en"], st["pq"]

                # logits[b,i,j] = q~_i . e_j  (DVE 1x dots, fp32 accumulator)
                lg = small.tile([P, TQ, TK], F32, tag="lg", name=f"lg{t}")
                dump = scr.tile([P, D], F16, tag="dump", name=f"du{t}")
                for i in range(TQ):
                    for j in range(TK):
                        nc.vector.scalar_tensor_tensor(
                            out=dump,
                            in0=pqs[i], scalar=1.0, in1=en[:, j, :],
                            op0=BYP, op1=MUL,
                            accum_out=lg[:, i, j:j + 1])

                nmx = small.tile([P, TQ], F32, tag="nmx", name=f"nm{t}")
                nc.vector.tensor_reduce(
                    out=nmx, in_=lg, axis=mybir.AxisListType.X,
                    op=mybir.AluOpType.max, negate=True)
                pr = small.tile([P, TQ, TK], F32, tag="pr", name=f"pr{t}")
                sm = small.tile([P, TQ], F32, tag="sm", name=f"sm{t}")
                for i in range(TQ):
                    nc.scalar.activation(
                        out=pr[:, i, :], in_=lg[:, i, :],
                        func=mybir.ActivationFunctionType.Exp,
                        bias=nmx[:, i:i + 1],
                        accum_out=sm[:, i:i + 1])
                st.update(pr=pr, sm=sm)
                return st

            def stage_c1(st):
                t, en, pr, sm = st["t"], st["en"], st["pr"], st["sm"]

                rs = small.tile([P, TQ], F32, tag="rs", name=f"rs{t}")
                nc.vector.reciprocal(rs, sm)

                # diag(ex_ij) = ident * ex_ij (i=0 j<4 on DVE, rest on Act;
                # with the Act qn-evac gone there is headroom for the full
                # 10-matmul diag on PE, so the old DVE accumulate hack is
                # dropped).
                dg = work.tile([P, TQ, TK, P], F16, tag="dg", name=f"dg{t}")
                for j in range(TK - 1):
                    nc.vector.tensor_scalar_mul(
                        dg[:, 0, j, :], ident, pr[:, 0, j:j + 1])
                nc.scalar.mul(dg[:, 0, TK - 1, :], ident, pr[:, 0, TK - 1:TK])
                for j in range(TK):
                    nc.scalar.mul(dg[:, 1, j, :], ident, pr[:, 1, j:j + 1])

                # ctxu_i = sum_j diag(ex_ij) @ e_j   (PE, PSUM accumulation)
                # normalize during PSUM->SBUF copy: ctx_i = ctxu_i * (1/sum_i)
                cx = work.tile([P, TQ, D], F16, tag="cx", name=f"cx{t}")
                for i in range(TQ):
                    pc = psc.tile([P, D], F32, tag="pc", name=f"pc{t}_{i}")
                    for j in range(TK):
                        nc.tensor.matmul(
                            pc, dg[:, i, j, :], en[:, j, :],
                            start=(j == 0), stop=(j == TK - 1))
                    nc.scalar.mul(cx[:, i, :], pc, rs[:, i:i + 1])
                st["cx"] = cx
                return st

            def stage_c2(st):
                t, hT, cx = st["t"], st["hT"], st["cx"]
                bsl = slice(t * P, (t + 1) * P)

                # transpose ctx -> cT [P(d), DC, TQ, P(b)]
                pt = pst.tile([P, TQ * DC, P], F16, tag="pt", name=f"pt{t}")
                for i in range(TQ):
                    for c in range(DC):
                        nc.tensor.transpose(
                            pt[:, i * DC + c, :],
                            cx[:, i, c * P:(c + 1) * P], ident)
                cT = work.tile([P, DC, TQ, P], F16, tag="cT", name=f"cT{t}")
                nc.scalar.copy(cT, pt.rearrange("p (i c) b -> p c i b", i=TQ))

                # out_i = h_i @ Wd1 + ctx_i @ Wvd; evac+store per i so the
                # final store isn't serialized behind both chains (tail)
                ob = obp.tile([P, TQ, D], F16, tag="ob", name=f"ob{t}")
                for i in range(TQ):
                    po = pso.tile([P, D], F32, tag="po", name=f"po{t}_{i}")
                    for c in range(DC):
                        nc.tensor.matmul(po, hT[:, c, i, :], wd1[:, c, :],
                                         start=(c == 0), stop=False)
                    for c in range(DC):
                        nc.tensor.matmul(po, cT[:, c, i, :], wvd[:, c, :],
                                         start=False, stop=(c == DC - 1))
                    nc.scalar.copy(ob[:, i, :], po)
                    nc.sync.dma_start(out=o_r[bsl, i, :], in_=ob[:, i, :])

            # Emit C1 before B each step: C1's small DVE ops (recip, diag
            # builds) feed this step's PE diag matmuls; queueing them ahead
            # of B's 6us dot block removes a PE stall window.
            stA, stB, stC = {}, {}, {}
            for tt in range(NT + 3):
                if tt < NT:
                    stA[tt] = stage_a(tt)
                if 2 <= tt < NT + 2:
                    stC[tt - 2] = stage_c1(stB.pop(tt - 2))
                if 1 <= tt < NT + 1:
                    stB[tt - 1] = stage_b(stA.pop(tt - 1))
                if tt >= 3:
                    stage_c2(stC.pop(tt - 3))

    nc.compile()
    return nc


def _pack_w(w):
    # [D, D] fp32 -> [P, DC, D] fp16 with contraction dim on partitions
    return np.ascontiguousarray(
        w.reshape(DC, P, D).transpose(1, 0, 2).astype(np.float16))


def kernel(h, enc_out, Wq, Wk, Wv, Wdown, _trace=False):
    h = np.ascontiguousarray(h, dtype=np.float32)
    enc_out = np.ascontiguousarray(enc_out, dtype=np.float32)
    Wq = np.ascontiguousarray(Wq, dtype=np.float32)
    Wk = np.ascontiguousarray(Wk, dtype=np.float32)
    Wv = np.ascontiguousarray(Wv, dtype=np.float32)
    Wdown = np.ascontiguousarray(Wdown, dtype=np.float32)

    if "nc" not in _CACHED:
        _CACHED["nc"] = build()
    nc = _CACHED["nc"]

    w_a = _pack_w(Wq @ Wk.T)
    w_d1 = _pack_w(Wdown[:D])
    w_vd = _pack_w(Wv @ Wdown[D:])

    h16 = h.astype(np.float16)
    e16 = enc_out.astype(np.float16)
    e_bm = np.ascontiguousarray(e16.transpose(1, 0, 2))        # [B, TK, D]
    # block-transposed lhsT tiles: [core][t][p(d%128)][c][i][p(b%128)]
    hT_bm = np.ascontiguousarray(
        h16.reshape(TQ, NCORES, NT, P, DC, P).transpose(1, 2, 5, 4, 0, 3))
    in_maps = []
    for c in range(NCORES):
        sl = slice(c * BL, (c + 1) * BL)
        in_maps.append({
            "enc": e_bm[sl],
            "hT": hT_bm[c],
            "Wqk": w_a, "Wd1": w_d1, "Wvd": w_vd,
        })

    res = run_bass_kernel_spmd(nc, in_maps, list(range(NCORES)), trace=_trace)
    out_bm = np.concatenate([r["out"] for r in res.results], axis=0)  # [B, TQ, D]
    out = np.ascontiguousarray(out_bm.transpose(1, 0, 2))
    if _trace:
        kernel.last_result = res
    return out.astype(np.float32)

